# revision 22
# baseline (speedup 1.0000x reference)
"""AHGNN hypergraph-conv kernel for 8 TRN2 NeuronCores.

Sharding: core c handles batch b = c//2, N-half h = c%2 (8192 points).
Hyperedge aggregation (E = H^T xf / deg_e) is partial per N-half and
AllReduced over core pairs; BatchNorm stats are AllReduced over all 8.
Output is produced directly in [C, N] layout per core.

The PE stream is software-pipelined one step: score/xf matmuls of tile i
run while the DVE top-k of tile i-1 finishes; H^T transposes + E-agg
matmuls of tile i-1 follow. This also keeps every matmul at <=1 new
semaphore wait (walrus LDWEIGHTS has a single sync-wait slot).
"""

import sys

sys.path.insert(0, "/opt/trn_rl_repo")

import numpy as np

import concourse.bass as bass
import concourse.bacc as bacc
import concourse.mybir as mybir
import concourse.tile as tile
from concourse.tile_rust import add_dep_helper

B, N, M, C, K = 4, 16384, 512, 256, 24
NCORES = 8
NS = N // 2  # points per core
EPS = 1e-5

f32 = mybir.dt.float32
bf16 = mybir.dt.bfloat16
ALU = mybir.AluOpType
ACT = mybir.ActivationFunctionType


def build_nc(ns=NS, n_total=B * N):
    nt = ns // 128
    nc = bacc.Bacc("TRN2", target_bir_lowering=False, debug=False)

    x_d = nc.declare_dram_parameter("x", [ns, C], f32, isOutput=False)
    # geo = [coords^T + ones row | anchors^T + (-.5||a||^2) row], one DMA
    geo_d = nc.declare_dram_parameter("geo", [4, ns + M], f32, isOutput=False)
    wt_d = nc.declare_dram_parameter("wt", [C, C], f32, isOutput=False)
    fb_d = nc.declare_dram_parameter("fb", [1, C], f32, isOutput=False)
    gm_d = nc.declare_dram_parameter("gm", [C, 1], f32, isOutput=False)
    bt_d = nc.declare_dram_parameter("bt", [C, 1], f32, isOutput=False)
    id_d = nc.declare_dram_parameter("ident", [128, 128], f32, isOutput=False)
    out_d = nc.declare_dram_parameter("out", [C, ns], f32, isOutput=True)

    inv_bn = 1.0 / float(n_total)

    with tile.TileContext(nc) as tc:
        with (
            tc.tile_pool(name="const", bufs=1) as cpool,
            tc.tile_pool(name="big", bufs=1) as bigpool,
            tc.tile_pool(name="dram", bufs=1, space="DRAM") as dpool,
            tc.tile_pool(name="small", bufs=1) as spool,
        ):
            # ---------------- constants / setup ----------------
            geo_sb = cpool.tile([4, ns + M], f32, tag="geo")
            nc.sync.dma_start(geo_sb[:, :], geo_d[:, :])

            ident_sb = cpool.tile([128, 128], f32, tag="ident")
            nc.sync.dma_start(ident_sb[:, :], id_d[:, :])

            wt_f = cpool.tile([128, 2 * C], f32, tag="wtf")
            nc.sync.dma_start(wt_f[:, 0:C], wt_d[0:128, :])
            nc.sync.dma_start(wt_f[:, C : 2 * C], wt_d[128:256, :])

            fb_sb = cpool.tile([1, C], f32, tag="fb")
            nc.sync.dma_start(fb_sb[:, :], fb_d[:, :])

            gm_sb = cpool.tile([128, 2], f32, tag="gm")
            nc.sync.dma_start(gm_sb[:, 0:1], gm_d[0:128, :])
            nc.sync.dma_start(gm_sb[:, 1:2], gm_d[128:256, :])
            bt_sb = cpool.tile([128, 2], f32, tag="bt")
            nc.sync.dma_start(bt_sb[:, 0:1], bt_d[0:128, :])
            nc.sync.dma_start(bt_sb[:, 1:2], bt_d[128:256, :])

            # casts & constants on ScalarE (keeps PE waits single-source)
            ib_sb = cpool.tile([128, 128], bf16, tag="ib")
            nc.scalar.copy(ib_sb[:, :], ident_sb[:, :])
            wt_bf = cpool.tile([128, 2 * C], bf16, tag="wtb")
            nc.scalar.copy(wt_bf[:, :], wt_f[:, :])
            ones_row = cpool.tile([1, 128], f32, tag="ones")
            nc.gpsimd.memset(ones_row[:, :], 1.0)

            # big persistent tensors
            HT_sb = bigpool.tile([128, nt * 512], bf16, tag="ht")  # [m, n] per (i, mc)
            yT_sb = bigpool.tile([128, 2 * ns], f32, tag="yt")  # [c, n] per half

            sumy = spool.tile([128, 2 * nt], f32, tag="sumy")
            sqy = spool.tile([128, 2 * nt], f32, tag="sqy")
            E_sb = spool.tile([128, 4 * (C + 1)], f32, tag="esb")

            # ---------------- phase 1 ----------------
            with (
                tc.tile_pool(name="pe", bufs=1, space="PSUM") as pe,
                tc.tile_pool(name="ps1", bufs=1, space="PSUM") as ps1,
                tc.tile_pool(name="work1", bufs=2) as w1,
                tc.tile_pool(name="work1b", bufs=2) as w1b,
            ):
                E_ps = [
                    pe.tile([128, C + 1], f32, tag=f"e{mc}", name=f"e{mc}")
                    for mc in range(4)
                ]
                # single PSUM tensors, rewritten every iteration (same-tensor
                # WAW on PE needs no semaphore; pool-slot cycling would add
                # PE self-waits and overflow walrus' LDW sync-wait slot)
                s_ps = ps1.tile([128, M], f32, tag="sps", name="sps")
                xt_ps = ps1.tile([128, C], f32, tag="xtps", name="xtps")
                xf_ps = ps1.tile([128, C], f32, tag="xfps", name="xfps")
                ht_ps = ps1.tile([128, M], bf16, tag="htps", name="htps")

                # absorber: observe ident/fb DMA ticks on PE before real matmuls
                nc.tensor.transpose(s_ps[:, 0:128], ident_sb[:, :], ident_sb[:, :])
                nc.tensor.transpose(s_ps[0:128, 128:129], fb_sb[0:1, 0:128], ident_sb[0:1, 0:1])

                # persistent xf_aug buffers: ones column written once (HW SBUF
                # is uninitialized; a scale=0 Copy would read real garbage)
                xfb2 = [
                    w1b.tile([128, C + 1], bf16, tag=f"xfb{k}", name=f"xfb{k}")
                    for k in range(2)
                ]
                for k in range(2):
                    nc.gpsimd.memset(xfb2[k][:, C : C + 1], 1.0)

                # fc_b broadcast to 128 partitions, scaled by 1/deg_v (= 1/K)
                nc.tensor.matmul(xf_ps[:, :], ones_row[:, :], fb_sb[:, :], start=True, stop=True)
                b24 = cpool.tile([128, C], f32, tag="b24")
                nc.scalar.activation(b24[:, :], xf_ps[:, :], ACT.Copy, scale=1.0 / K)

                state = {}  # tiles of in-flight pipeline step

                def emit_front(i):
                    """score matmul + x transpose + xf matmuls for tile i."""
                    csl = slice(i * 128, (i + 1) * 128)
                    s_mm = nc.tensor.matmul(
                        s_ps[:, :], geo_sb[:, csl], geo_sb[:, ns : ns + M],
                        start=True, stop=True,
                    )
                    x_sb = w1.tile([128, C], f32, tag="xsb", name=f"x{i}")
                    nc.sync.dma_start(x_sb[:, :], x_d[csl, :])
                    # keep the transpose after s_mm in the schedule: s_mm's ACT
                    # wait (s_copy WAR) then covers xt_ps' ACT WAR, so the
                    # transpose carries only its single DMA wait (LDW slot limit)
                    xt_a = nc.tensor.transpose(xt_ps[:, 0:128], x_sb[:, 0:128], ident_sb[:, :])
                    add_dep_helper(xt_a.ins, s_mm.ins, sync=False, reason="ldw-wait-slot")
                    nc.tensor.transpose(xt_ps[:, 128:256], x_sb[:, 128:256], ident_sb[:, :])

                    # ScalarE ladder: xt_copy, then s_copy
                    xt_bf = w1b.tile([128, C], bf16, tag="xtbf", name=f"xtb{i}")
                    nc.scalar.copy(xt_bf[:, :], xt_ps[:, :])
                    s_sb = w1.tile([128, M], f32, tag="ssb", name=f"s{i}")
                    nc.scalar.copy(s_sb[:, :], s_ps[:, :])
                    nc.tensor.matmul(
                        xf_ps[:, :], xt_bf[:, 0:128], wt_bf[:, 0:C], start=True, stop=False
                    )
                    nc.tensor.matmul(
                        xf_ps[:, :], xt_bf[:, 128:256], wt_bf[:, C : 2 * C],
                        start=False, stop=True,
                    )
                    xf_bf = xfb2[i % 2]
                    nc.scalar.copy(xf_bf[:, 0:C], xf_ps[:, :])

                    # DVE top-24 chain
                    m8a = w1.tile([128, 8], f32, tag="m8a", name=f"m8a{i}")
                    m8b = w1.tile([128, 8], f32, tag="m8b", name=f"m8b{i}")
                    m8c = w1.tile([128, 8], f32, tag="m8c", name=f"m8c{i}")
                    s2 = w1.tile([128, M], f32, tag="s2", name=f"s2_{i}")
                    s3 = w1.tile([128, M], f32, tag="s3", name=f"s3_{i}")
                    nc.vector.max(m8a[:, :], s_sb[:, :])
                    nc.vector.match_replace(s2[:, :], m8a[:, :], s_sb[:, :], -1e30)
                    nc.vector.max(m8b[:, :], s2[:, :])
                    nc.vector.match_replace(s3[:, :], m8b[:, :], s2[:, :], -1e30)
                    nc.vector.max(m8c[:, :], s3[:, :])
                    H_sb = w1.tile([128, M], bf16, tag="hsb", name=f"h{i}")
                    nc.vector.tensor_scalar(
                        H_sb[:, :], s_sb[:, :], m8c[:, 7:8], None, ALU.is_ge
                    )
                    state[i] = (H_sb, xf_bf)

                def emit_back(i):
                    """H^T transposes + E-agg matmuls for tile i."""
                    H_sb, xf_bf = state.pop(i)
                    for mc in range(4):
                        nc.tensor.transpose(
                            ht_ps[:, mc * 128 : (mc + 1) * 128],
                            H_sb[:, mc * 128 : (mc + 1) * 128],
                            ib_sb[:, :],
                        )
                    for mc in range(4):
                        nc.tensor.matmul(
                            E_ps[mc][:, :],
                            H_sb[:, mc * 128 : (mc + 1) * 128],
                            xf_bf[:, :],
                            start=(i == 0),
                            stop=(i == nt - 1),
                        )
                    nc.scalar.copy(HT_sb[:, i * 512 : (i + 1) * 512], ht_ps[:, :])

                emit_front(0)
                for i in range(1, nt):
                    emit_front(i)
                    emit_back(i - 1)
                emit_back(nt - 1)

                for mc in range(4):
                    nc.scalar.copy(
                        E_sb[:, mc * (C + 1) : (mc + 1) * (C + 1)], E_ps[mc][:, :]
                    )
                # retire each E bank with a PE write (1 ACT wait each) so the
                # banks' release deps are PE-only; phase 2's first writers then
                # carry at most one foreign wait (walrus LDW slot limit)
                for mc in (3, 2, 1, 0):
                    nc.tensor.transpose(
                        E_ps[mc][:, 0:128], ident_sb[:, :], ident_sb[:, :]
                    )

            e_loc = dpool.tile([128, 4 * (C + 1)], f32, tag="eloc")
            e_red = dpool.tile([128, 4 * (C + 1)], f32, tag="ered")
            nc.sync.dma_start(e_loc[:, :], E_sb[:, :])
            nc.gpsimd.collective_compute(
                "AllReduce",
                ALU.add,
                replica_groups=[[0, 1], [2, 3], [4, 5], [6, 7]],
                ins=[e_loc[:, :].opt()],
                outs=[e_red[:, :].opt()],
            )
            E2_sb = spool.tile([128, 4 * (C + 1)], f32, tag="e2sb")
            nc.sync.dma_start(E2_sb[:, :], e_red[:, :])

            # E_used = (E_num * inv_deg + fc_b) / 24   (bf16)
            Eu_bf = spool.tile([128, 4 * C], bf16, tag="eubf")
            Eu_f = spool.tile([128, C], f32, tag="euf")
            inv24 = spool.tile([128, 4], f32, tag="inv24")
            for mc in range(4):
                dg = E2_sb[:, mc * (C + 1) + C : mc * (C + 1) + C + 1]
                nc.vector.tensor_scalar(
                    inv24[:, mc : mc + 1], dg, 0.5, float(K), ALU.max, ALU.mult
                )
                nc.vector.reciprocal(inv24[:, mc : mc + 1], inv24[:, mc : mc + 1])
                nc.vector.tensor_scalar(
                    Eu_f[:, :],
                    E2_sb[:, mc * (C + 1) : mc * (C + 1) + C],
                    inv24[:, mc : mc + 1],
                    None,
                    ALU.mult,
                )
                nc.vector.tensor_tensor(
                    Eu_bf[:, mc * C : (mc + 1) * C], Eu_f[:, :], b24[:, :], ALU.add
                )

            # ---------------- phase 2: y = H @ E_used + x ----------------
            with (
                tc.tile_pool(name="ps2", bufs=1, space="PSUM") as ps2,
                tc.tile_pool(name="work2", bufs=3) as w2,
            ):
                y_ps2 = [
                    ps2.tile([128, C], f32, tag=f"yps{k}", name=f"yps{k}")
                    for k in range(2)
                ]
                yt_ps2 = [
                    ps2.tile([128, C], f32, tag=f"ytps{k}", name=f"ytps{k}")
                    for k in range(2)
                ]
                scr2 = ps2.tile([128, 256], bf16, tag="yscr", name="scr2")
                # absorbers: PSUM bank-release PE tick, last HT ScalarE tick,
                # Eu DVE tick — one foreign wait per PE instruction
                nc.tensor.transpose(scr2[:, 0:128], ib_sb[:, :], ib_sb[:, :])
                nc.tensor.transpose(
                    scr2[:, 0:128],
                    HT_sb[:, (nt - 1) * 512 : (nt - 1) * 512 + 128],
                    ib_sb[:, :],
                )
                nc.tensor.transpose(scr2[:, 128:256], Eu_bf[:, 0:128], ib_sb[:, :])

                ystate = {}
                yt_insts = {}

                def emit_y(i):
                    csl = slice(i * 128, (i + 1) * 128)
                    x2_sb = w2.tile([128, C], f32, tag="x2sb", name=f"x2_{i}")
                    nc.sync.dma_start(x2_sb[:, :], x_d[csl, :])
                    y_ps = y_ps2[i % 2]
                    resid = nc.tensor.matmul(
                        y_ps[:, :], ident_sb[:, :], x2_sb[:, :], start=True, stop=False
                    )
                    if i - 2 in yt_insts:
                        # order after yt transpose(i-2) whose ACT wait covers
                        # this matmul's y_ps WAR (same buffer parity)
                        add_dep_helper(resid.ins, yt_insts[i - 2].ins, sync=False, reason="ldw-wait-slot")
                    for mc in range(4):
                        nc.tensor.matmul(
                            y_ps[:, :],
                            HT_sb[:, i * 512 + mc * 128 : i * 512 + (mc + 1) * 128],
                            Eu_bf[:, mc * C : (mc + 1) * C],
                            start=False,
                            stop=(mc == 3),
                        )
                    y_sb = w2.tile([128, C], f32, tag="ysb", name=f"ys{i}")
                    nc.scalar.copy(y_sb[:, :], y_ps[:, :])
                    ystate[i] = y_sb

                def emit_yt(i):
                    y_sb = ystate.pop(i)
                    yt_ps = yt_ps2[i % 2]
                    yt_a = nc.tensor.transpose(yt_ps[:, 0:128], y_sb[:, 0:128], ident_sb[:, :])
                    yt_insts[i] = yt_a
                    nc.tensor.transpose(yt_ps[:, 128:256], y_sb[:, 128:256], ident_sb[:, :])
                    sq_scr = w2.tile([128, 128], f32, tag="sqscr", name=f"sq{i}")
                    for hf in range(2):
                        nc.scalar.activation(
                            yT_sb[:, hf * ns + i * 128 : hf * ns + (i + 1) * 128],
                            yt_ps[:, hf * 128 : (hf + 1) * 128],
                            ACT.Copy,
                            accum_out=sumy[:, hf * nt + i : hf * nt + i + 1],
                        )
                        nc.scalar.activation(
                            sq_scr[:, :],
                            yt_ps[:, hf * 128 : (hf + 1) * 128],
                            ACT.Square,
                            accum_out=sqy[:, hf * nt + i : hf * nt + i + 1],
                        )

                emit_y(0)
                for i in range(1, nt):
                    emit_y(i)
                    emit_yt(i - 1)
                emit_yt(nt - 1)

            # ---------------- BN stats allreduce + affine ----------------
            st_sb = spool.tile([128, 4], f32, tag="stsb")
            nc.vector.tensor_reduce(st_sb[:, 0:1], sumy[:, 0:nt], axis=mybir.AxisListType.X, op=ALU.add)
            nc.vector.tensor_reduce(st_sb[:, 1:2], sqy[:, 0:nt], axis=mybir.AxisListType.X, op=ALU.add)
            nc.vector.tensor_reduce(st_sb[:, 2:3], sumy[:, nt : 2 * nt], axis=mybir.AxisListType.X, op=ALU.add)
            nc.vector.tensor_reduce(st_sb[:, 3:4], sqy[:, nt : 2 * nt], axis=mybir.AxisListType.X, op=ALU.add)

            st_loc = dpool.tile([128, 4], f32, tag="stloc")
            st_red = dpool.tile([128, 4], f32, tag="stred")
            nc.sync.dma_start(st_loc[:, :], st_sb[:, :])
            nc.gpsimd.collective_compute(
                "AllReduce",
                ALU.add,
                replica_groups=[list(range(NCORES))],
                ins=[st_loc[:, :].opt()],
                outs=[st_red[:, :].opt()],
            )
            st2 = spool.tile([128, 4], f32, tag="st2")
            nc.sync.dma_start(st2[:, :], st_red[:, :])

            scale_c = spool.tile([128, 2], f32, tag="scalec")
            shift_c = spool.tile([128, 2], f32, tag="shiftc")
            mu = spool.tile([128, 2], f32, tag="mu")
            var = spool.tile([128, 2], f32, tag="var")
            tmp = spool.tile([128, 2], f32, tag="tmpc")
            for hf in range(2):
                nc.vector.tensor_scalar(mu[:, hf : hf + 1], st2[:, 2 * hf : 2 * hf + 1], inv_bn, None, ALU.mult)
                nc.vector.tensor_scalar(var[:, hf : hf + 1], st2[:, 2 * hf + 1 : 2 * hf + 2], inv_bn, None, ALU.mult)
                nc.vector.tensor_tensor(tmp[:, hf : hf + 1], mu[:, hf : hf + 1], mu[:, hf : hf + 1], ALU.mult)
                nc.vector.tensor_tensor(var[:, hf : hf + 1], var[:, hf : hf + 1], tmp[:, hf : hf + 1], ALU.subtract)
            eps_col = spool.tile([128, 1], f32, tag="epsc")
            nc.gpsimd.memset(eps_col[:, :], EPS)
            nc.scalar.activation(var[:, :], var[:, :], ACT.Sqrt, bias=eps_col[:, :])
            nc.vector.reciprocal(var[:, :], var[:, :])
            nc.vector.tensor_tensor(scale_c[:, :], gm_sb[:, :], var[:, :], ALU.mult)
            nc.vector.tensor_tensor(tmp[:, :], mu[:, :], scale_c[:, :], ALU.mult)
            nc.vector.tensor_tensor(shift_c[:, :], bt_sb[:, :], tmp[:, :], ALU.subtract)

            # ---------------- phase 3: silu(scale*yT + shift) -> out ----------------
            with tc.tile_pool(name="work3", bufs=4) as w3:
                nj = ns // 512
                for hf in range(2):
                    for j in range(nj):
                        z = w3.tile([128, 512], f32, tag="z", name=f"z{hf}_{j}")
                        nc.scalar.activation(
                            z[:, :],
                            yT_sb[:, hf * ns + j * 512 : hf * ns + (j + 1) * 512],
                            ACT.Silu,
                            bias=shift_c[:, hf : hf + 1],
                            scale=scale_c[:, hf : hf + 1],
                        )
                        nc.sync.dma_start(
                            out_d[hf * 128 : (hf + 1) * 128, j * 512 : (j + 1) * 512],
                            z[:, :],
                        )

    nc.compile()
    return nc


_NC_CACHE = {}


def _in_maps(x, coords, anchors, fc_w, fc_b, bn_gamma, bn_beta, ns=NS):
    wt = np.ascontiguousarray(fc_w.T).astype(np.float32)
    ident = np.eye(128, dtype=np.float32)
    fb = np.ascontiguousarray(fc_b.reshape(1, C)).astype(np.float32)
    gm = np.ascontiguousarray(bn_gamma.reshape(C, 1)).astype(np.float32)
    bt = np.ascontiguousarray(bn_beta.reshape(C, 1)).astype(np.float32)
    maps = []
    for c in range(NCORES):
        b, h = c // 2, c % 2
        sl = slice(h * ns, (h + 1) * ns)
        ca = np.concatenate([coords[b, sl].T, np.ones((1, ns), np.float32)], axis=0)
        aa = np.concatenate(
            [anchors[b].T, -0.5 * np.sum(anchors[b] ** 2, -1)[None, :]], axis=0
        )
        geo = np.concatenate([ca, aa], axis=1).astype(np.float32)
        maps.append(
            {
                "x": np.ascontiguousarray(x[b, sl]).astype(np.float32),
                "geo": np.ascontiguousarray(geo),
                "wt": wt,
                "fb": fb,
                "gm": gm,
                "bt": bt,
                "ident": ident,
            }
        )
    return maps


def kernel(x, coords, anchors, fc_w, fc_b, bn_gamma, bn_beta):
    from concourse.bass_utils import run_bass_kernel_spmd

    x = np.asarray(x, np.float32)
    coords = np.asarray(coords, np.float32)
    anchors = np.asarray(anchors, np.float32)
    fc_w = np.asarray(fc_w, np.float32)
    fc_b = np.asarray(fc_b, np.float32)
    bn_gamma = np.asarray(bn_gamma, np.float32)
    bn_beta = np.asarray(bn_beta, np.float32)

    if "nc" not in _NC_CACHE:
        _NC_CACHE["nc"] = build_nc()
    nc = _NC_CACHE["nc"]
    maps = _in_maps(x, coords, anchors, fc_w, fc_b, bn_gamma, bn_beta)
    res = run_bass_kernel_spmd(nc, maps, core_ids=list(range(NCORES)))
    outs = res.results
    full = np.zeros((B, C, N), np.float32)
    for c in range(NCORES):
        b, h = c // 2, c % 2
        full[b, :, h * NS : (h + 1) * NS] = outs[c]["out"]
    return full
